# revision 1
# baseline (speedup 1.0000x reference)
"""Trainium2 Bass kernel: single-head causal attention, data-parallel over batch.

Problem: x [4096, 64, 128] f32, Wq/Wk/Wv [128, 64] f32.
  q,k,v = x @ W*;  scores = q k^T / sqrt(128); causal softmax; out = attn @ v.

Sharding: batch 4096 -> 8 cores x 512 batches. Each core loops over 32
super-tiles of 16 batches (1024 rows of x).

Per-core pipeline (bf16 matmuls, fp32 PSUM):
  1. SWDGE DMA-cast loads x tile [128, 1024] f32->bf16.
  2. 8 PE transposes -> x^T in PSUM (bf16) -> SBUF.
  3. P1: q^T,k^T = W^T x^T per batch column-blocks (parity -> partition half).
  4. P2: v pairs in native [s, h] layout (x^T pair as stationary).
  5. P3: scores^T_b = k_b q_b^T per batch into quadrant-packed PSUM.
  6. exp on ACT (PSUM->SBUF bf16), multiplicative causal mask on GPSIMD.
  7. P4: [O'|sums] = E^T.T @ [V|ones] per batch (unnormalized attn out).
  8. normalize: O = O' * recip(sums) via stride-0 broadcast tensor_tensor.
"""

import os
import numpy as np
import ml_dtypes
from contextlib import ExitStack

F32 = None  # set after imports below (keep module import cheap for host-only use)

B, T, C, H = 4096, 64, 128, 64
N_CORES = 8
ST_B = 16                    # batches per super-tile
ROWS = ST_B * T              # 1024
B_CORE = B // N_CORES        # 512
N_ST = B_CORE // ST_B        # 32

_cached = {}


def _build_nc():
    import concourse.bass as bass
    import concourse.mybir as mybir
    import concourse.tile as tile
    from concourse import bacc

    F32 = mybir.dt.float32
    BF16 = mybir.dt.bfloat16

    nc = bacc.Bacc("TRN2", target_bir_lowering=False, debug=False)
    x_d = nc.dram_tensor("x", [B_CORE * T, C], F32, kind="ExternalInput").ap()
    wq_d = nc.dram_tensor("wq", [C, H], BF16, kind="ExternalInput").ap()
    wk_d = nc.dram_tensor("wk", [C, H], BF16, kind="ExternalInput").ap()
    wv_d = nc.dram_tensor("wv", [C, H], BF16, kind="ExternalInput").ap()
    id_d = nc.dram_tensor("ident", [C, C], BF16, kind="ExternalInput").ap()
    mk_d = nc.dram_tensor("mask", [128, 512], BF16, kind="ExternalInput").ap()
    o_d = nc.dram_tensor("o", [B_CORE * T, H], F32, kind="ExternalOutput").ap()

    with tile.TileContext(nc) as tc, ExitStack() as ctx:
        sb = ctx.enter_context(tc.tile_pool(name="sb", bufs=2))
        ps = ctx.enter_context(tc.tile_pool(name="ps", bufs=1, space="PSUM"))
        psO = ctx.enter_context(tc.tile_pool(name="psO", bufs=1, space="PSUM"))
        cpool = ctx.enter_context(tc.tile_pool(name="const", bufs=1))

        wq_sb = cpool.tile([C, H], BF16, tag="wq")
        wk_sb = cpool.tile([C, H], BF16, tag="wk")
        wv_sb = cpool.tile([C, H], BF16, tag="wv")
        id_sb = cpool.tile([C, C], BF16, tag="id")
        mk_sb = cpool.tile([128, 512], BF16, tag="mk")
        nc.sync.dma_start(wq_sb[:], wq_d)
        nc.sync.dma_start(wk_sb[:], wk_d)
        nc.sync.dma_start(wv_sb[:], wv_d)
        nc.sync.dma_start(id_sb[:], id_d)
        nc.sync.dma_start(mk_sb[:], mk_d)

        xv = x_d.rearrange("(S n p) c -> S p n c", n=8, p=128)
        ov = o_d.rearrange("(S m par t) h -> S (par t) m h", m=8, par=2, t=64)

        for st in range(N_ST):
            # ---- load x (f32), cast to bf16 on GPSIMD
            x_nat = sb.tile([128, ROWS], F32, tag="x_nat")
            nc.sync.dma_start(
                x_nat[:].rearrange("p (n c) -> p n c", n=8), xv[st]
            )
            x_bf = sb.tile([128, ROWS], BF16, tag="x_bf")
            nc.gpsimd.tensor_copy(x_bf[:], x_nat[:])

            # ---- 8 PE transposes -> xT in PSUM (bf16), then copy to SBUF
            xT_ps = ps.tile([128, ROWS // 2], F32, tag="xT")
            xT_ps_bf = xT_ps[:].bitcast(BF16)
            for i in range(8):
                nc.tensor.transpose(
                    xT_ps_bf[:, 128 * i:128 * (i + 1)],
                    x_bf[:, 128 * i:128 * (i + 1)],
                    id_sb[:],
                )
            xT_sb = sb.tile([128, ROWS], BF16, tag="xT_sb")
            nc.vector.tensor_copy(xT_sb[:], xT_ps_bf)

            # ---- P1: q^T, k^T
            # bank b (cols 512b): [0:64, 0:256]=q evens, [0:64, 256:512]=k evens
            #                     [64:128, ...] odds
            qk_ps = ps.tile([128, 1024], F32, tag="qk")
            xTv = xT_sb[:].rearrange("p (m par t) -> p par m t", par=2, t=64)
            for b in range(2):
                for par in range(2):
                    for qki, w_sb in ((0, wq_sb), (1, wk_sb)):
                        nc.tensor.matmul(
                            qk_ps[64 * par:64 * par + 64,
                                  512 * b + 256 * qki:512 * b + 256 * qki + 256],
                            w_sb[:],
                            xTv[:, par, 4 * b:4 * b + 4, :],
                            start=True, stop=True, skip_group_check=True,
                            tile_position=(0, 64 * par),
                        )
            qk_sb = sb.tile([128, 1024], BF16, tag="qk_sb")
            nc.scalar.copy(qk_sb[:], qk_ps[:])

            # ---- P2: v pairs (native [s,h])
            v_ps = ps.tile([128, 512], F32, tag="v")
            for m in range(8):
                nc.tensor.matmul(
                    v_ps[:, 64 * m:64 * m + 64],
                    xT_sb[:, 128 * m:128 * m + 128],
                    wv_sb[:],
                    start=True, stop=True,
                )
            v_sb = sb.tile([128, 8 * 66], BF16, tag="v_sb")
            v_sb_v = v_sb[:].rearrange("p (m z) -> p m z", z=66)
            nc.vector.tensor_copy(
                v_sb_v[:, :, 0:64],
                v_ps[:].rearrange("p (m t) -> p m t", t=64),
            )
            nc.gpsimd.memset(v_sb_v[:, :, 64:65], 1.0)

            # ---- P3: scores^T per batch (quadrant-packed pairs)
            sc_ps = ps.tile([128, 512], F32, tag="sc")
            for j in range(16):
                m, Hh = j // 2, 64 * (j % 2)
                b, e = j // 8, (j % 8) // 2
                qcol = 512 * b + 64 * e
                kcol = 512 * b + 256 + 64 * e
                nc.tensor.matmul(
                    sc_ps[Hh:Hh + 64, 64 * m:64 * m + 64],
                    qk_sb[Hh:Hh + 64, kcol:kcol + 64],
                    qk_sb[Hh:Hh + 64, qcol:qcol + 64],
                    start=True, stop=True, skip_group_check=True,
                    tile_position=(Hh, Hh),
                )

            # ---- exp (ACT) then multiplicative causal mask (GPSIMD)
            E_raw = sb.tile([128, 512], BF16, tag="Eraw")
            nc.scalar.activation(
                E_raw[:], sc_ps[:], mybir.ActivationFunctionType.Exp
            )
            E_sb = sb.tile([128, 512], BF16, tag="E")
            nc.gpsimd.tensor_tensor(
                out=E_sb[:], in0=E_raw[:], in1=mk_sb[:],
                op=mybir.AluOpType.mult,
            )

            # ---- P4: [O' | sums] per batch
            o_ps = psO.tile([128, 1024], F32, tag="o")
            for j in range(16):
                m, Hh = j // 2, 64 * (j % 2)
                off = 512 * (m // 4) + 65 * (m % 4)
                nc.tensor.matmul(
                    o_ps[Hh:Hh + 64, off:off + 65],
                    E_sb[Hh:Hh + 64, 64 * m:64 * m + 64],
                    v_sb[Hh:Hh + 64, 66 * m:66 * m + 65],
                    start=True, stop=True, skip_group_check=True,
                    tile_position=(Hh, Hh),
                )

            # ---- normalize: O = O' * recip(sums)
            opsv = o_ps[:].rearrange("p (B x) -> p B x", B=2)[:, :, 0:260]
            opsb = opsv.rearrange("p B (m z) -> p B m z", z=65)
            r_sb = sb.tile([128, 8], F32, tag="r")
            r_v = r_sb[:].rearrange("p (B m) -> p B m", B=2)
            nc.vector.reciprocal(r_v.unsqueeze(3), opsb[:, :, :, 64:65])
            o_sb = sb.tile([128, 512], F32, tag="o_sb")
            nc.vector.tensor_tensor(
                out=o_sb[:].rearrange("p (B m t) -> p B m t", B=2, t=64),
                in0=opsb[:, :, :, 0:64],
                in1=r_v.unsqueeze(3).broadcast_to((128, 2, 4, 64)),
                op=mybir.AluOpType.mult,
            )

            # ---- DMA out
            nc.sync.dma_start(ov[st], o_sb[:].rearrange("p (m h) -> p m h", h=64))

    nc.compile()
    return nc


def _host_inputs(x, Wq, Wk, Wv):
    bf = ml_dtypes.bfloat16
    wq_bf = np.ascontiguousarray((Wq * (C ** -0.5)).astype(bf))
    wk_bf = np.ascontiguousarray(Wk.astype(bf))
    wv_bf = np.ascontiguousarray(Wv.astype(bf))
    ident = np.eye(128, dtype=bf)
    tri = np.triu(np.ones((T, T), dtype=np.float32))  # [s, t]: 1 if s <= t
    mask = np.ascontiguousarray(np.tile(tri, (2, 8)).astype(bf))
    in_maps = []
    for c in range(N_CORES):
        shard = np.ascontiguousarray(
            x[c * B_CORE:(c + 1) * B_CORE].reshape(B_CORE * T, C)
        ).astype(np.float32)
        in_maps.append({
            "x": shard, "wq": wq_bf, "wk": wk_bf, "wv": wv_bf,
            "ident": ident, "mask": mask,
        })
    return in_maps


def run(x, Wq, Wk, Wv, trace=False, **run_kwargs):
    from concourse import bass_utils

    if "nc" not in _cached:
        _cached["nc"] = _build_nc()
    nc = _cached["nc"]
    in_maps = _host_inputs(np.asarray(x), np.asarray(Wq),
                           np.asarray(Wk), np.asarray(Wv))
    res = bass_utils.run_bass_kernel_spmd(
        nc, in_maps, core_ids=list(range(N_CORES)), trace=trace, **run_kwargs
    )
    outs = [r["o"].reshape(B_CORE, T, H) for r in res.results]
    return np.concatenate(outs, axis=0), res


def kernel(x, Wq, Wk, Wv):
    out, _ = run(x, Wq, Wk, Wv, trace=False)
    return out



# revision 2
# speedup vs baseline: 1.5588x; 1.5588x over previous
"""Trainium2 Bass kernel: single-head causal attention, data-parallel over batch.

Problem: x [4096, 64, 128] f32, Wq/Wk/Wv [128, 64] f32.
  q,k,v = x @ W*;  scores = q k^T / sqrt(128); causal softmax; out = attn @ v.

Sharding: batch 4096 -> 8 cores x 512 batches. Each core loops over 32
super-tiles of 16 batches (1024 rows of x).

Key restructuring vs a naive q/k/v pipeline:
  * A-trick: scores = x A x^T with A = Wq Wk^T / sqrt(C) folded on host.
    One [128x128] stationary (A^T) + 2 big matmuls per tile replace the
    whole q/k projection stage.
  * SWDGE cast-DMA loads x f32->bf16 straight from HBM (no engine cast).
  * Batch-pair packing: each pair of batches shares one 128-row block.
    P_S computes a [128,128] block = diag(scores_e^T, scores_o^T) plus
    off-diagonal garbage in ONE matmul; the causal mask (kron(I2, tri))
    zeroes the garbage after exp. P4 then uses diag(E_e, E_o) as a single
    128-col stationary to produce both batches' [O'|sums] in ONE matmul.

Per-core pipeline (bf16 matmuls, fp32 PSUM):
  1. SWDGE DMA-cast x tile [128, 1024] f32->bf16.
  2. 8 PE transposes -> x^T (PSUM, bf16) -> SBUF (vector).
  3. Y = A x^T: 2 matmuls N=512 (stationary A^T) -> PSUM -> SBUF bf16 (scalar).
  4. v = x W~v: 8 matmuls (stationary x^T chunks, rhs wv) -> SBUF bf16 + ones col.
  5. P_S: 8 pair-matmuls -> sc_ps [128, 1024] (diag-packed scores^T).
  6. exp on ACT (PSUM->SBUF bf16), mask-mult on vector (zeroes garbage).
  7. P4: 8 pair-matmuls [O'|sums] = E^T.T @ [V|ones].
  8. normalize: O = O' * recip(sums); DMA out.
"""

import os
import numpy as np
import ml_dtypes
from contextlib import ExitStack

B, T, C, H = 4096, 64, 128, 64
N_CORES = 8
ST_B = 16                    # batches per super-tile
ROWS = ST_B * T              # 1024
B_CORE = B // N_CORES        # 512
N_ST = B_CORE // ST_B        # 32

_cached = {}


def _build_nc():
    import concourse.bass as bass
    import concourse.mybir as mybir
    import concourse.tile as tile
    from concourse import bacc

    F32 = mybir.dt.float32
    BF16 = mybir.dt.bfloat16

    nc = bacc.Bacc("TRN2", target_bir_lowering=False, debug=False)
    x_d = nc.dram_tensor("x", [B_CORE * T, C], F32, kind="ExternalInput").ap()
    at_d = nc.dram_tensor("at", [C, C], BF16, kind="ExternalInput").ap()
    wv_d = nc.dram_tensor("wv", [C, H], BF16, kind="ExternalInput").ap()
    id_d = nc.dram_tensor("ident", [C, C], BF16, kind="ExternalInput").ap()
    mk_d = nc.dram_tensor("mask", [128, 1024], BF16, kind="ExternalInput").ap()
    o_d = nc.dram_tensor("o", [B_CORE * T, H], F32, kind="ExternalOutput").ap()

    with tile.TileContext(nc) as tc, ExitStack() as ctx:
        sb = ctx.enter_context(tc.tile_pool(name="sb", bufs=2))
        ps = ctx.enter_context(tc.tile_pool(name="ps", bufs=1, space="PSUM"))
        psO = ctx.enter_context(tc.tile_pool(name="psO", bufs=1, space="PSUM"))
        cpool = ctx.enter_context(tc.tile_pool(name="const", bufs=1))

        at_sb = cpool.tile([C, C], BF16, tag="at")
        wv_sb = cpool.tile([C, H], BF16, tag="wv")
        id_sb = cpool.tile([C, C], BF16, tag="id")
        mk_sb = cpool.tile([128, 1024], BF16, tag="mk")
        nc.sync.dma_start(at_sb[:], at_d)
        nc.sync.dma_start(wv_sb[:], wv_d)
        nc.sync.dma_start(id_sb[:], id_d)
        nc.sync.dma_start(mk_sb[:], mk_d)

        xv = x_d.rearrange("(S n p) c -> S p n c", n=8, p=128)
        ov = o_d.rearrange("(S m par t) h -> S (par t) m h", m=8, par=2, t=64)

        for st in range(N_ST):
            # ---- SWDGE cast-load x (f32 HBM -> bf16 SBUF)
            x_bf = sb.tile([128, ROWS], BF16, tag="x_bf")
            nc.gpsimd.dma_start(
                x_bf[:].rearrange("p (n c) -> p n c", n=8), xv[st]
            )

            # ---- 8 PE transposes -> xT in PSUM (bf16), then copy to SBUF
            xT_ps = ps.tile([128, ROWS // 2], F32, tag="xT")
            xT_ps_bf = xT_ps[:].bitcast(BF16)
            for i in range(8):
                nc.tensor.transpose(
                    xT_ps_bf[:, 128 * i:128 * (i + 1)],
                    x_bf[:, 128 * i:128 * (i + 1)],
                    id_sb[:],
                )
            xT_sb = sb.tile([128, ROWS], BF16, tag="xT_sb")
            nc.vector.tensor_copy(xT_sb[:], xT_ps_bf)

            # ---- Y = A x^T (stationary A^T), 2 matmuls N=512
            y_ps = ps.tile([128, 1024], F32, tag="y")
            for half in range(2):
                nc.tensor.matmul(
                    y_ps[:, 512 * half:512 * half + 512],
                    at_sb[:],
                    xT_sb[:, 512 * half:512 * half + 512],
                    start=True, stop=True,
                )
            y_sb = sb.tile([128, 1024], BF16, tag="y_sb")
            nc.scalar.copy(y_sb[:], y_ps[:])

            # ---- v = x @ wv (stationary x^T chunks, rhs wv)
            v_ps = ps.tile([128, 512], F32, tag="v")
            for m in range(8):
                nc.tensor.matmul(
                    v_ps[:, 64 * m:64 * m + 64],
                    xT_sb[:, 128 * m:128 * m + 128],
                    wv_sb[:],
                    start=True, stop=True,
                )
            v_sb = sb.tile([128, 8 * 66], BF16, tag="v_sb")
            v_sb_v = v_sb[:].rearrange("p (m z) -> p m z", z=66)
            nc.vector.tensor_copy(
                v_sb_v[:, :, 0:64],
                v_ps[:].rearrange("p (m t) -> p m t", t=64),
            )
            nc.gpsimd.memset(v_sb_v[:, :, 64:65], 1.0)

            # ---- P_S: diag-packed scores^T per batch pair, ONE matmul each
            sc_ps = ps.tile([128, 1024], F32, tag="sc")
            for m in range(8):
                nc.tensor.matmul(
                    sc_ps[:, 128 * m:128 * m + 128],
                    y_sb[:, 128 * m:128 * m + 128],
                    xT_sb[:, 128 * m:128 * m + 128],
                    start=True, stop=True,
                )

            # ---- exp (ACT) then multiplicative causal mask (vector)
            E_raw = sb.tile([128, 1024], BF16, tag="Eraw")
            nc.scalar.activation(
                E_raw[:], sc_ps[:], mybir.ActivationFunctionType.Exp
            )
            E_sb = sb.tile([128, 1024], BF16, tag="E")
            nc.vector.tensor_tensor(
                out=E_sb[:], in0=E_raw[:], in1=mk_sb[:],
                op=mybir.AluOpType.mult,
            )

            # ---- P4: [O' | sums] per batch pair, ONE matmul each
            o_ps = psO.tile([128, 1024], F32, tag="o")
            for m in range(8):
                off = 512 * (m // 4) + 65 * (m % 4)
                nc.tensor.matmul(
                    o_ps[:, off:off + 65],
                    E_sb[:, 128 * m:128 * m + 128],
                    v_sb[:, 66 * m:66 * m + 65],
                    start=True, stop=True,
                )

            # ---- normalize: O = O' * recip(sums)
            opsv = o_ps[:].rearrange("p (B x) -> p B x", B=2)[:, :, 0:260]
            opsb = opsv.rearrange("p B (m z) -> p B m z", z=65)
            r_sb = sb.tile([128, 8], F32, tag="r")
            r_v = r_sb[:].rearrange("p (B m) -> p B m", B=2)
            nc.vector.reciprocal(r_v.unsqueeze(3), opsb[:, :, :, 64:65])
            o_sb = sb.tile([128, 512], F32, tag="o_sb")
            nc.vector.tensor_tensor(
                out=o_sb[:].rearrange("p (B m t) -> p B m t", B=2, t=64),
                in0=opsb[:, :, :, 0:64],
                in1=r_v.unsqueeze(3).broadcast_to((128, 2, 4, 64)),
                op=mybir.AluOpType.mult,
            )

            # ---- DMA out
            nc.sync.dma_start(ov[st], o_sb[:].rearrange("p (m h) -> p m h", h=64))

    nc.compile()
    return nc


def _host_inputs(x, Wq, Wk, Wv):
    bf = ml_dtypes.bfloat16
    at = np.ascontiguousarray((Wk @ Wq.T * (C ** -0.5)).astype(bf))
    wv_bf = np.ascontiguousarray(Wv.astype(bf))
    ident = np.eye(128, dtype=bf)
    tri = np.triu(np.ones((T, T), dtype=np.float32))  # [s, t]: 1 if s <= t
    mask_pair = np.kron(np.eye(2, dtype=np.float32), tri)  # [128, 128]
    mask = np.ascontiguousarray(np.tile(mask_pair, (1, 8)).astype(bf))
    in_maps = []
    for c in range(N_CORES):
        shard = np.ascontiguousarray(
            x[c * B_CORE:(c + 1) * B_CORE].reshape(B_CORE * T, C)
        ).astype(np.float32)
        in_maps.append({
            "x": shard, "at": at, "wv": wv_bf,
            "ident": ident, "mask": mask,
        })
    return in_maps


def run(x, Wq, Wk, Wv, trace=False, **run_kwargs):
    from concourse import bass_utils

    if "nc" not in _cached:
        _cached["nc"] = _build_nc()
    nc = _cached["nc"]
    in_maps = _host_inputs(np.asarray(x), np.asarray(Wq),
                           np.asarray(Wk), np.asarray(Wv))
    res = bass_utils.run_bass_kernel_spmd(
        nc, in_maps, core_ids=list(range(N_CORES)), trace=trace, **run_kwargs
    )
    outs = [r["o"].reshape(B_CORE, T, H) for r in res.results]
    return np.concatenate(outs, axis=0), res


def kernel(x, Wq, Wk, Wv):
    out, _ = run(x, Wq, Wk, Wv, trace=False)
    return out
